# revision 1
# baseline (speedup 1.0000x reference)
"""MMD loss kernel for Trainium2 (8 NeuronCores, Bass/Tile).

reference math:
  src = X[:2048], tgt = X[2048:],  D=512
  xx = mean over [4096,4096] of sum_k exp(-d2_dup(src,src)/(bw_xx*2^k))
  (dup matrix mean == mean over the 2048^2 block), similarly yy, and
  xy uses the full 4096^2 matrix of X.
  bw for (a,b) = sum(d2([a;b]))/(m^2-m) / mul^(num//2),  mul=2, num=5.

Strategy:
  - bandwidth sums have a closed form: sum_block d2 = 2n*sum(sq) - 2|sum x|^2
    -> computed host-side in fp64, passed to the device as runtime
    activation *scales* (per-partition AP), so no first pass over d2.
  - pairwise tile: PSUM M = G - sq_i/2 - sq_j/2 = -d2/2 via an augmented
    matmul (K=512 data + K=4 aug rows with bf16 hi/lo split of -sq/2).
  - 5-kernel sum: u = exp(scale*M) with scale = 1/(8*bw_base); then 4
    squarings give the other 4 kernels. Every pass carries an accum_out
    rider = per-partition row sum, so no separate reductions.
  - symmetry: the distance matrix is symmetric. Own-half blocks use cyclic
    coverage (each 512-row core covers col-groups k,k+1,k+2 with weights
    1,2,1); cross src/tgt blocks are covered once with weight 2 across the
    8 cores. Every core runs the SAME program on a per-core permuted
    column layout: local cols = [own(k), own(k+1), own(k+2), cross0, cross1]
    (2560 of 4096 columns).
"""

import sys

sys.path.insert(0, "/opt/trn_rl_repo")

import numpy as np
import ml_dtypes

N, D, HALF, BLK = 4096, 512, 2048, 512
NCORES = 8
NSTRIP = 4          # 4 strips of 128 rows per core
NCHUNK = 5          # local col chunks of 512: 3 own (w 1,2,1) + 2 cross (w 2)
CHUNK_W = [1, 2, 1, 2, 2]
NPASS = 5           # exp + 4 squares
RID_W = 5           # rider slots per unit

# squares engine pattern per chain: pass i on DVE if SQ_ON_DVE[i]
SQ_ON_DVE = [True, False, True, False]
MM_DT = "bfloat16"


def _schedule():
    """Static (core-independent) unit schedule: (chunk, chain)."""
    sched = []
    for c in range(NCHUNK):
        chains = ("own", "xy") if c < 3 else ("xy",)
        for chain in chains:
            sched.append((c, chain))
    return sched


SCHED = _schedule()
NUNIT = len(SCHED)  # 8
REPEAT = 1


def _local_cols(core):
    half, k = core // 4, core % 4
    own_base, other_base = half * HALF, (1 - half) * HALF
    groups = [k, (k + 1) % 4, (k + 2) % 4]
    cols = [own_base + 512 * g + np.arange(512) for g in groups]
    if half == 0:
        cross = [0, 1] if k % 2 == 0 else [2, 3]
    else:
        cross = [1, 3] if k < 2 else [0, 2]
    cols += [other_base + 512 * b + np.arange(512) for b in cross]
    return np.concatenate(cols)


def _build_program():
    import concourse.bacc as bacc
    import concourse.mybir as mybir
    import concourse.tile as tile

    f32 = mybir.dt.float32
    mm_dt = getattr(mybir.dt, MM_DT)
    LC = NCHUNK * 512  # 2560 local columns

    nc = bacc.Bacc("TRN2", target_bir_lowering=False, debug=False,
                   num_devices=NCORES)
    xth_d = nc.dram_tensor("xth", [4, 128, LC], mm_dt, kind="ExternalInput")
    xtl_d = nc.dram_tensor("xtl", [4, 128, LC], mm_dt, kind="ExternalInput")
    aug_d = nc.dram_tensor("aug", [4, LC + 512], mm_dt, kind="ExternalInput")
    sc_d = nc.dram_tensor("scales", [128, 2], f32, kind="ExternalInput")
    nrep = globals().get("REPEAT", 1)
    rid_d = nc.dram_tensor("riders", [nrep * NUNIT, 128, RID_W], f32,
                           kind="ExternalOutput")

    with tile.TileContext(nc) as tc:
        with (
            tc.tile_pool(name="xtp", bufs=1) as xtp,
            tc.tile_pool(name="augp", bufs=1) as augp,
            tc.tile_pool(name="scp", bufs=1) as scp,
            tc.tile_pool(name="ridp", bufs=1) as ridp,
            tc.tile_pool(name="psp", bufs=8, space="PSUM") as psp,
            tc.tile_pool(name="up", bufs=4) as up,
        ):
            xth = [xtp.tile([128, LC], mm_dt, tag=f"xth{k}", name=f"xth{k}")
                   for k in range(4)]
            xtl = [xtp.tile([128, LC], mm_dt, tag=f"xtl{k}", name=f"xtl{k}")
                   for k in range(4)]
            aug = augp.tile([4, LC + 512], mm_dt, tag="aug", name="aug")
            sc = scp.tile([128, 2], f32, tag="sc", name="sc")
            for k in range(4):
                nc.sync.dma_start(out=xth[k][:], in_=xth_d.ap()[k])
                nc.sync.dma_start(out=xtl[k][:], in_=xtl_d.ap()[k])
            nc.sync.dma_start(out=aug[:], in_=aug_d.ap())
            nc.sync.dma_start(out=sc[:], in_=sc_d.ap())

            riders = [[ridp.tile([128, RID_W], f32, tag=f"rid{u}_{rp}",
                                 name=f"rid{u}_{rp}") for u in range(NUNIT)]
                      for rp in range(nrep)]

            by_chunk = {}
            for u, (c, chain) in enumerate(SCHED):
                by_chunk.setdefault(c, []).append((u, chain))

            for rep in range(nrep):
                for c, chains in sorted(by_chunk.items()):
                    ps = psp.tile([128, 2048], f32, tag="ps", name="ps", bufs=2)
                    for s in range(4):
                        pss = ps[:, 512 * s:512 * s + 512]
                        for k in range(4):
                            lh = xth[k][:, 128 * s:128 * s + 128]
                            ll = xtl[k][:, 128 * s:128 * s + 128]
                            rh = xth[k][:, 512 * c:512 * c + 512]
                            rl = xtl[k][:, 512 * c:512 * c + 512]
                            nc.tensor.matmul(out=pss, lhsT=lh, rhs=rh,
                                             start=(k == 0), stop=False)
                            nc.tensor.matmul(out=pss, lhsT=lh, rhs=rl,
                                             start=False, stop=False)
                            nc.tensor.matmul(out=pss, lhsT=ll, rhs=rh,
                                             start=False, stop=False)
                        nc.tensor.matmul(
                            out=pss,
                            lhsT=aug[:, LC + 128 * s:LC + 128 * s + 128],
                            rhs=aug[:, 512 * c:512 * c + 512],
                            start=False, stop=True)

                    if globals().get("SKIP_CHAINS", False):
                        for u, chain in chains:
                            nc.vector.tensor_reduce(
                                out=riders[rep][u][:, 0:1], in_=ps[:, 0:512],
                                axis=mybir.AxisListType.X,
                                op=mybir.AluOpType.add)
                            nc.vector.tensor_copy(
                                riders[rep][u][:, 1:RID_W],
                                ps[:, 0:RID_W - 1])
                        continue
                    for u, chain in chains:
                        rid = riders[rep][u]
                        sci = 0 if chain == "own" else 1
                        cur = up.tile([128, 2048], f32, tag="u", name="u", bufs=2)
                        nc.scalar.activation(
                            out=cur[:], in_=ps[:],
                            func=mybir.ActivationFunctionType.Exp,
                            scale=sc[:, sci:sci + 1],
                            accum_out=rid[:, 0:1])
                        for p in range(4):
                            nxt = up.tile([128, 2048], f32, tag=f"u{p}",
                                          name=f"u{p}", bufs=2)
                            if SQ_ON_DVE[p]:
                                nc.vector.scalar_tensor_tensor(
                                    out=nxt[:], in0=cur[:], scalar=1.0,
                                    in1=cur[:],
                                    op0=mybir.AluOpType.mult,
                                    op1=mybir.AluOpType.mult,
                                    accum_out=rid[:, p + 1:p + 2])
                            else:
                                nc.scalar.activation(
                                    out=nxt[:], in_=cur[:],
                                    func=mybir.ActivationFunctionType.Square,
                                    accum_out=rid[:, p + 1:p + 2])
                            cur = nxt

            for rp in range(nrep):
                for u in range(NUNIT):
                    nc.sync.dma_start(out=rid_d.ap()[rp * NUNIT + u],
                                      in_=riders[rp][u][:])

    nc.compile()
    return nc


_PROG = None


def _get_program():
    global _PROG
    if _PROG is None:
        _PROG = _build_program()
    return _PROG


def _prep_inputs(latent):
    X = np.asarray(latent, np.float32)
    X64 = X.astype(np.float64)
    sq = (X64 * X64).sum(1)                      # [N]
    M2 = float(N) * N - N

    def block_d2_sum(lo, hi):
        n = hi - lo
        sv = X64[lo:hi].sum(0)
        return 2.0 * (n * sq[lo:hi].sum()) - 2.0 * (sv @ sv)

    S_src = block_d2_sum(0, HALF)
    S_tgt = block_d2_sum(HALF, N)
    sv_all = X64.sum(0)
    S_full = 2.0 * (N * sq.sum()) - 2.0 * (sv_all @ sv_all)

    bw_xx = S_src / M2           # already includes /mul^(num//2) (see notes)
    bw_yy = S_tgt / M2
    bw_xy = (S_full / M2) / 4.0

    in_maps = []
    for core in range(NCORES):
        lc = _local_cols(core)
        xf = X[lc].T.reshape(4, 128, NCHUNK * 512)
        xth = np.ascontiguousarray(xf).astype(ml_dtypes.bfloat16)
        xtl = np.ascontiguousarray(
            xf - xth.astype(np.float32)).astype(ml_dtypes.bfloat16)
        sql = sq[lc]
        v = -0.5 * sql
        hi = np.asarray(v, ml_dtypes.bfloat16).astype(np.float64)
        lo = (v - hi).astype(np.float32)
        hi = hi.astype(np.float32)
        ones = np.ones_like(hi)
        aug = np.zeros((4, NCHUNK * 512 + 512), ml_dtypes.bfloat16)
        aug[0, :NCHUNK * 512] = hi
        aug[1, :NCHUNK * 512] = lo
        aug[2, :NCHUNK * 512] = ones
        aug[3, :NCHUNK * 512] = ones
        aug[0, NCHUNK * 512:] = 1.0
        aug[1, NCHUNK * 512:] = 1.0
        aug[2, NCHUNK * 512:] = hi[:512]
        aug[3, NCHUNK * 512:] = lo[:512]

        bw_own = bw_xx if core < 4 else bw_yy
        scales = np.zeros((128, 2), np.float32)
        scales[:, 0] = 1.0 / (8.0 * bw_own)
        scales[:, 1] = 1.0 / (8.0 * bw_xy)
        in_maps.append({"xth": xth, "xtl": xtl, "aug": aug,
                        "scales": scales})
    return in_maps


def _postprocess(results):
    S_own = np.zeros(NCORES)
    S_xy = np.zeros(NCORES)
    for core in range(NCORES):
        r = results[core]["riders"].astype(np.float64)  # [NUNIT,128,RID_W]
        for u, (c, chain) in enumerate(SCHED):
            val = CHUNK_W[c] * r[u, :, :NPASS].sum()
            if chain == "own":
                S_own[core] += val
            else:
                S_xy[core] += val
    xx = S_own[:4].sum() / (HALF * HALF)
    yy = S_own[4:].sum() / (HALF * HALF)
    xy = S_xy.sum() / (float(N) * N)
    return np.float32(xx + yy - 2.0 * xy)


def _run(inputs, trace=False, **kw):
    from concourse.bass_utils import run_bass_kernel_spmd
    nc = _get_program()
    in_maps = _prep_inputs(inputs["latent"])
    res = run_bass_kernel_spmd(nc, in_maps, list(range(NCORES)),
                               trace=trace, **kw)
    return _postprocess(res.results), res


def kernel(**inputs):
    out, _ = _run(inputs, trace=False)
    return out


if __name__ == "__main__":
    rng = np.random.default_rng(0)
    lat = rng.standard_normal((N, D)).astype(np.float32)
    print(kernel(latent=lat,
                 domain=np.concatenate([np.zeros(HALF, np.int32),
                                        np.ones(HALF, np.int32)])))



# revision 3
# speedup vs baseline: 44.5952x; 44.5952x over previous
"""MMD loss kernel for Trainium2 (8 NeuronCores, Bass/Tile).

reference math:
  src = X[:2048], tgt = X[2048:],  D=512
  xx = mean over [4096,4096] of sum_k exp(-d2_dup(src,src)/(bw_xx*2^k))
  (dup matrix mean == mean over the 2048^2 block), similarly yy, and
  xy uses the full 4096^2 matrix of X.
  bw for (a,b) = sum(d2([a;b]))/(m^2-m) / mul^(num//2),  mul=2, num=5.

Strategy:
  - bandwidth sums have a closed form: sum_block d2 = 2n*sum(sq) - 2|sum x|^2
    -> computed host-side in fp64, passed to the device as runtime
    activation *scales* (per-partition AP), so no first pass over d2.
  - pairwise tile: PSUM M = G - sq_i/2 - sq_j/2 = -d2/2 via an augmented
    matmul (K=512 data + K=4 aug rows with bf16 hi/lo split of -sq/2).
  - 5-kernel sum: u = exp(scale*M) with scale = 1/(8*bw_base); then 4
    squarings give the other 4 kernels. Every pass carries an accum_out
    rider = per-partition row sum, so no separate reductions.
  - symmetry: the distance matrix is symmetric. Own-half blocks use cyclic
    coverage (each 512-row core covers col-groups k,k+1,k+2 with weights
    1,2,1); cross src/tgt blocks are covered once with weight 2 across the
    8 cores. Every core runs the SAME program on a per-core permuted
    column layout: local cols = [own(k), own(k+1), own(k+2), cross0, cross1]
    (2560 of 4096 columns).
"""

import sys

sys.path.insert(0, "/opt/trn_rl_repo")

import numpy as np
import ml_dtypes

N, D, HALF, BLK = 4096, 512, 2048, 512
NCORES = 8
NSTRIP = 4          # 4 strips of 128 rows per core
NCHUNK = 5          # local col chunks of 512: 3 own (w 1,2,1) + 2 cross (w 2)
CHUNK_W = [1, 2, 1, 2, 2]
NPASS = 5           # exp + 4 squares
RID_W = 5           # rider slots per unit

# squares engine pattern per chain: pass i on DVE if SQ_ON_DVE[i]
SQ_ON_DVE = [True, False, True, False]
MM_DT = "bfloat16"


def _schedule():
    """Static (core-independent) unit schedule: (chunk, chain)."""
    sched = []
    for c in range(NCHUNK):
        chains = ("own", "xy") if c < 3 else ("xy",)
        for chain in chains:
            sched.append((c, chain))
    return sched


SCHED = _schedule()
NUNIT = len(SCHED)  # 8
REPEAT = 1


def _local_cols(core):
    half, k = core // 4, core % 4
    own_base, other_base = half * HALF, (1 - half) * HALF
    groups = [k, (k + 1) % 4, (k + 2) % 4]
    cols = [own_base + 512 * g + np.arange(512) for g in groups]
    if half == 0:
        cross = [0, 1] if k % 2 == 0 else [2, 3]
    else:
        cross = [1, 3] if k < 2 else [0, 2]
    cols += [other_base + 512 * b + np.arange(512) for b in cross]
    return np.concatenate(cols)


def _build_program():
    import concourse.bacc as bacc
    import concourse.mybir as mybir
    import concourse.tile as tile

    f32 = mybir.dt.float32
    mm_dt = getattr(mybir.dt, MM_DT)
    LC = NCHUNK * 512  # 2560 local columns

    nc = bacc.Bacc("TRN2", target_bir_lowering=False, debug=False,
                   num_devices=NCORES)
    xth_d = nc.dram_tensor("xth", [4, 128, LC], mm_dt, kind="ExternalInput")
    xtl_d = nc.dram_tensor("xtl", [4, 128, LC], mm_dt, kind="ExternalInput")
    aug_d = nc.dram_tensor("aug", [4, LC + 512], mm_dt, kind="ExternalInput")
    sc_d = nc.dram_tensor("scales", [128, 2], f32, kind="ExternalInput")
    nrep = globals().get("REPEAT", 1)
    # Output only the final rep's riders: keeps the output tensor size
    # independent of REPEAT so the wall-delta timing isolates device time
    # (otherwise the tunnel transfer of the extra output dominates).
    rid_d = nc.dram_tensor("riders", [NUNIT, 128, RID_W], f32,
                           kind="ExternalOutput")

    with tile.TileContext(nc) as tc:
        with (
            tc.tile_pool(name="xtp", bufs=1) as xtp,
            tc.tile_pool(name="augp", bufs=1) as augp,
            tc.tile_pool(name="scp", bufs=1) as scp,
            tc.tile_pool(name="ridp", bufs=1) as ridp,
            tc.tile_pool(name="psp", bufs=8, space="PSUM") as psp,
            tc.tile_pool(name="up", bufs=4) as up,
        ):
            xth = [xtp.tile([128, LC], mm_dt, tag=f"xth{k}", name=f"xth{k}")
                   for k in range(4)]
            xtl = [xtp.tile([128, LC], mm_dt, tag=f"xtl{k}", name=f"xtl{k}")
                   for k in range(4)]
            aug = augp.tile([4, LC + 512], mm_dt, tag="aug", name="aug")
            sc = scp.tile([128, 2], f32, tag="sc", name="sc")
            for k in range(4):
                nc.sync.dma_start(out=xth[k][:], in_=xth_d.ap()[k])
                nc.sync.dma_start(out=xtl[k][:], in_=xtl_d.ap()[k])
            nc.sync.dma_start(out=aug[:], in_=aug_d.ap())
            nc.sync.dma_start(out=sc[:], in_=sc_d.ap())

            riders = [[ridp.tile([128, RID_W], f32, tag=f"rid{u}_{rp}",
                                 name=f"rid{u}_{rp}") for u in range(NUNIT)]
                      for rp in range(nrep)]

            by_chunk = {}
            for u, (c, chain) in enumerate(SCHED):
                by_chunk.setdefault(c, []).append((u, chain))

            for rep in range(nrep):
                for c, chains in sorted(by_chunk.items()):
                    ps = psp.tile([128, 2048], f32, tag="ps", name="ps", bufs=2)
                    for s in range(4):
                        pss = ps[:, 512 * s:512 * s + 512]
                        for k in range(4):
                            lh = xth[k][:, 128 * s:128 * s + 128]
                            ll = xtl[k][:, 128 * s:128 * s + 128]
                            rh = xth[k][:, 512 * c:512 * c + 512]
                            rl = xtl[k][:, 512 * c:512 * c + 512]
                            nc.tensor.matmul(out=pss, lhsT=lh, rhs=rh,
                                             start=(k == 0), stop=False)
                            nc.tensor.matmul(out=pss, lhsT=lh, rhs=rl,
                                             start=False, stop=False)
                            nc.tensor.matmul(out=pss, lhsT=ll, rhs=rh,
                                             start=False, stop=False)
                        nc.tensor.matmul(
                            out=pss,
                            lhsT=aug[:, LC + 128 * s:LC + 128 * s + 128],
                            rhs=aug[:, 512 * c:512 * c + 512],
                            start=False, stop=True)

                    if globals().get("SKIP_CHAINS", False):
                        for u, chain in chains:
                            nc.vector.tensor_reduce(
                                out=riders[rep][u][:, 0:1], in_=ps[:, 0:512],
                                axis=mybir.AxisListType.X,
                                op=mybir.AluOpType.add)
                            nc.vector.tensor_copy(
                                riders[rep][u][:, 1:RID_W],
                                ps[:, 0:RID_W - 1])
                        continue
                    for u, chain in chains:
                        rid = riders[rep][u]
                        sci = 0 if chain == "own" else 1
                        cur = up.tile([128, 2048], f32, tag="u", name="u", bufs=2)
                        nc.scalar.activation(
                            out=cur[:], in_=ps[:],
                            func=mybir.ActivationFunctionType.Exp,
                            scale=sc[:, sci:sci + 1],
                            accum_out=rid[:, 0:1])
                        for p in range(4):
                            nxt = up.tile([128, 2048], f32, tag=f"u{p}",
                                          name=f"u{p}", bufs=2)
                            if SQ_ON_DVE[p]:
                                nc.vector.scalar_tensor_tensor(
                                    out=nxt[:], in0=cur[:], scalar=1.0,
                                    in1=cur[:],
                                    op0=mybir.AluOpType.mult,
                                    op1=mybir.AluOpType.mult,
                                    accum_out=rid[:, p + 1:p + 2])
                            else:
                                nc.scalar.activation(
                                    out=nxt[:], in_=cur[:],
                                    func=mybir.ActivationFunctionType.Square,
                                    accum_out=rid[:, p + 1:p + 2])
                            cur = nxt

            for u in range(NUNIT):
                nc.sync.dma_start(out=rid_d.ap()[u],
                                  in_=riders[nrep - 1][u][:])

    nc.compile()
    return nc


_PROG = None


def _get_program():
    global _PROG
    if _PROG is None:
        _PROG = _build_program()
    return _PROG


def _prep_inputs(latent):
    X = np.asarray(latent, np.float32)
    X64 = X.astype(np.float64)
    sq = (X64 * X64).sum(1)                      # [N]
    M2 = float(N) * N - N

    def block_d2_sum(lo, hi):
        n = hi - lo
        sv = X64[lo:hi].sum(0)
        return 2.0 * (n * sq[lo:hi].sum()) - 2.0 * (sv @ sv)

    S_src = block_d2_sum(0, HALF)
    S_tgt = block_d2_sum(HALF, N)
    sv_all = X64.sum(0)
    S_full = 2.0 * (N * sq.sum()) - 2.0 * (sv_all @ sv_all)

    bw_xx = S_src / M2           # already includes /mul^(num//2) (see notes)
    bw_yy = S_tgt / M2
    bw_xy = (S_full / M2) / 4.0

    in_maps = []
    for core in range(NCORES):
        lc = _local_cols(core)
        xf = X[lc].T.reshape(4, 128, NCHUNK * 512)
        xth = np.ascontiguousarray(xf).astype(ml_dtypes.bfloat16)
        xtl = np.ascontiguousarray(
            xf - xth.astype(np.float32)).astype(ml_dtypes.bfloat16)
        sql = sq[lc]
        v = -0.5 * sql
        hi = np.asarray(v, ml_dtypes.bfloat16).astype(np.float64)
        lo = (v - hi).astype(np.float32)
        hi = hi.astype(np.float32)
        ones = np.ones_like(hi)
        aug = np.zeros((4, NCHUNK * 512 + 512), ml_dtypes.bfloat16)
        aug[0, :NCHUNK * 512] = hi
        aug[1, :NCHUNK * 512] = lo
        aug[2, :NCHUNK * 512] = ones
        aug[3, :NCHUNK * 512] = ones
        aug[0, NCHUNK * 512:] = 1.0
        aug[1, NCHUNK * 512:] = 1.0
        aug[2, NCHUNK * 512:] = hi[:512]
        aug[3, NCHUNK * 512:] = lo[:512]

        bw_own = bw_xx if core < 4 else bw_yy
        scales = np.zeros((128, 2), np.float32)
        scales[:, 0] = 1.0 / (8.0 * bw_own)
        scales[:, 1] = 1.0 / (8.0 * bw_xy)
        in_maps.append({"xth": xth, "xtl": xtl, "aug": aug,
                        "scales": scales})
    return in_maps


def _postprocess(results):
    S_own = np.zeros(NCORES)
    S_xy = np.zeros(NCORES)
    for core in range(NCORES):
        r = results[core]["riders"].astype(np.float64)  # [NUNIT,128,RID_W]
        for u, (c, chain) in enumerate(SCHED):
            val = CHUNK_W[c] * r[u, :, :NPASS].sum()
            if chain == "own":
                S_own[core] += val
            else:
                S_xy[core] += val
    xx = S_own[:4].sum() / (HALF * HALF)
    yy = S_own[4:].sum() / (HALF * HALF)
    xy = S_xy.sum() / (float(N) * N)
    return np.float32(xx + yy - 2.0 * xy)


def _run(inputs, trace=False, **kw):
    from concourse.bass_utils import run_bass_kernel_spmd
    nc = _get_program()
    in_maps = _prep_inputs(inputs["latent"])
    res = run_bass_kernel_spmd(nc, in_maps, list(range(NCORES)),
                               trace=trace, **kw)
    return _postprocess(res.results), res


def kernel(**inputs):
    out, _ = _run(inputs, trace=False)
    return out


if __name__ == "__main__":
    rng = np.random.default_rng(0)
    lat = rng.standard_normal((N, D)).astype(np.float32)
    print(kernel(latent=lat,
                 domain=np.concatenate([np.zeros(HALF, np.int32),
                                        np.ones(HALF, np.int32)])))



# revision 11
# speedup vs baseline: 141.2880x; 3.1682x over previous
"""MMD loss kernel for Trainium2 (8 NeuronCores, Bass/Tile).

reference math:
  src = X[:2048], tgt = X[2048:],  D=512
  xx = mean over [4096,4096] of sum_k exp(-d2_dup(src,src)/(bw_xx*2^k))
  (dup matrix mean == mean over the 2048^2 block), similarly yy, and
  xy uses the full 4096^2 matrix of X.
  bw for (a,b) = sum(d2([a;b]))/(m^2-m) / mul^(num//2),  mul=2, num=5.

Strategy:
  - bandwidth sums have a closed form: sum_block d2 = 2n*sum(sq) - 2|sum x|^2
    -> computed host-side in fp64, passed to the device as runtime
    activation *scales* (per-partition AP), so no first pass over d2.
  - pairwise tile: PSUM M = G - sq_i/2 - sq_j/2 = -d2/2 via an augmented
    matmul (K=512 bf16 data + K=4 aug rows with bf16 hi/lo split of -sq/2).
  - 5-kernel sum: u = exp(scale*M) with scale = 1/(8*bw_base) on ACT
    (accum_out rider = sum u), then ONE fused custom-DVE op computes
    u^2+u^4+u^8+u^16 elementwise (8 ALU stages) with accum_out rider =
    its row sum. The host only needs the total of the 5 kernel sums, so
    two riders per chain suffice.
  - symmetry: the distance matrix is symmetric. Own-half blocks use cyclic
    coverage (each 512-row core covers col-groups k,k+1,k+2 with weights
    1,2,1); cross src/tgt blocks are covered once with weight 2 across the
    8 cores. Every core runs the SAME program on a per-core permuted
    column layout: local cols = [own(k), own(k+1), own(k+2), cross0, cross1]
    (2560 of 4096 columns).
  - timing contract: only the final rep's riders are DMA'd out, so the
    output size is independent of REPEAT and the wall-delta between
    REPEAT variants isolates on-device body time.
"""

import sys

sys.path.insert(0, "/opt/trn_rl_repo")

import numpy as np
import ml_dtypes

N, D, HALF, BLK = 4096, 512, 2048, 512
NCORES = 8
NSTRIP = 4          # 4 strips of 128 rows per core
NCHUNK = 5          # local col chunks of 512: 3 own (w 1,2,1) + 2 cross (w 2)
CHUNK_W = [1, 2, 1, 2, 2]
RID_W = 2           # rider slots per unit: [sum u, sum u^2+u^4+u^8+u^16]

MM_DT = "bfloat16"
U_DT = "float32"    # dtype of the exp output / fused-op scratch tiles


def _schedule():
    """Static (core-independent) unit schedule: (chunk, chain)."""
    sched = []
    for c in range(NCHUNK):
        chains = ("own", "xy") if c < 3 else ("xy",)
        for chain in chains:
            sched.append((c, chain))
    return sched


SCHED = _schedule()
NUNIT = len(SCHED)  # 8
REPEAT = 1


_MMD_OP = None


def _get_mmd_op():
    """Fused DVE op: out = u^2+u^4+u^8+u^16, accum_out = row-sum(out).

    Registered once into dve_ops.OPS (the sanctioned custom-DVE extension
    point; the uop table is emitted per-NEFF at compile time)."""
    global _MMD_OP
    if _MMD_OP is not None:
        return _MMD_OP
    from concourse import dve_ops
    from concourse.dve_spec import Spec, Src0, sq, lower
    from concourse.dve_uop import AluOp, DveOpSpec

    name = "MMD_POW_SUM"
    for op in dve_ops.OPS:
        if op.name == name:
            _MMD_OP = op
            return op

    a = sq(Src0)
    b = sq(a)
    c = sq(b)
    d = sq(c)

    def _ref(in0, in1, c0, c1, c2):
        x = in0.astype(np.float32)
        aa = x * x
        bb = aa * aa
        cc = bb * bb
        dd = cc * cc
        body = (aa + bb) + (cc + dd)
        return body, body.reshape(body.shape[0], -1).sum(
            axis=-1, keepdims=True)

    spec = Spec(body=(a + b) + (c + d), accum=AluOp.ADD, reference=_ref)
    row = max(dve_ops._SUB_OPCODE_FOR_NAME.values()) + 1
    assert row < 0x20, "custom-DVE opcode rows exhausted"
    shas = {}
    for ver in ("v3", "v4"):
        uops = lower(spec, ver=ver)
        shas[ver] = DveOpSpec(name=name, opcode=row, uops=uops,
                              rd1_en=False).sha(ver)
    op = dve_ops.DveOp(name, spec, subdim=False, uops_sha=shas)
    dve_ops.OPS.append(op)
    dve_ops._SUB_OPCODE_FOR_NAME[name] = row
    dve_ops.CUSTOM_DVE_SPECS[name] = spec
    _MMD_OP = op
    return op


def _local_cols(core):
    half, k = core // 4, core % 4
    own_base, other_base = half * HALF, (1 - half) * HALF
    groups = [k, (k + 1) % 4, (k + 2) % 4]
    cols = [own_base + 512 * g + np.arange(512) for g in groups]
    if half == 0:
        cross = [0, 1] if k % 2 == 0 else [2, 3]
    else:
        cross = [1, 3] if k < 2 else [0, 2]
    cols += [other_base + 512 * b + np.arange(512) for b in cross]
    return np.concatenate(cols)


def _build_program():
    import concourse.bacc as bacc
    import concourse.mybir as mybir
    import concourse.tile as tile

    f32 = mybir.dt.float32
    mm_dt = getattr(mybir.dt, MM_DT)
    u_dt = getattr(mybir.dt, U_DT)
    LC = NCHUNK * 512  # 2560 local columns
    mmd_op = _get_mmd_op()

    nc = bacc.Bacc("TRN2", target_bir_lowering=False, debug=False,
                   num_devices=NCORES)
    xth_d = nc.dram_tensor("xth", [4, 128, LC], mm_dt, kind="ExternalInput")
    aug_d = nc.dram_tensor("aug", [4, LC + 512], mm_dt, kind="ExternalInput")
    sc_d = nc.dram_tensor("scales", [128, 2], f32, kind="ExternalInput")
    nrep = globals().get("REPEAT", 1)
    # Output only the final rep's riders: keeps the output tensor size
    # independent of REPEAT so the wall-delta timing isolates device time
    # (otherwise the tunnel transfer of the extra output dominates).
    rid_d = nc.dram_tensor("riders", [NUNIT, 128, RID_W], f32,
                           kind="ExternalOutput")

    with tile.TileContext(nc) as tc:
        with (
            tc.tile_pool(name="xtp", bufs=1) as xtp,
            tc.tile_pool(name="augp", bufs=1) as augp,
            tc.tile_pool(name="scp", bufs=1) as scp,
            tc.tile_pool(name="ridp", bufs=1) as ridp,
            tc.tile_pool(name="psp", bufs=8, space="PSUM") as psp,
            tc.tile_pool(name="up", bufs=4) as up,
        ):
            xth = [xtp.tile([128, LC], mm_dt, tag=f"xth{k}", name=f"xth{k}")
                   for k in range(4)]
            aug = augp.tile([4, LC + 512], mm_dt, tag="aug", name="aug")
            sc = scp.tile([128, 2], f32, tag="sc", name="sc")
            for k in range(4):
                nc.sync.dma_start(out=xth[k][:], in_=xth_d.ap()[k])
            nc.sync.dma_start(out=aug[:], in_=aug_d.ap())
            nc.sync.dma_start(out=sc[:], in_=sc_d.ap())

            riders = [[ridp.tile([128, RID_W], f32, tag=f"rid{u}_{rp}",
                                 name=f"rid{u}_{rp}") for u in range(NUNIT)]
                      for rp in range(nrep)]

            by_chunk = {}
            for u, (c, chain) in enumerate(SCHED):
                by_chunk.setdefault(c, []).append((u, chain))

            for rep in range(nrep):
                for c, chains in sorted(by_chunk.items()):
                    ps = psp.tile([128, 2048], f32, tag="ps", name="ps", bufs=2)
                    for s in range(4):
                        pss = ps[:, 512 * s:512 * s + 512]
                        for k in range(4):
                            lh = xth[k][:, 128 * s:128 * s + 128]
                            rh = xth[k][:, 512 * c:512 * c + 512]
                            nc.tensor.matmul(out=pss, lhsT=lh, rhs=rh,
                                             start=(k == 0), stop=False)
                        nc.tensor.matmul(
                            out=pss,
                            lhsT=aug[:, LC + 128 * s:LC + 128 * s + 128],
                            rhs=aug[:, 512 * c:512 * c + 512],
                            start=False, stop=True)

                    for u, chain in chains:
                        rid = riders[rep][u]
                        sci = 0 if chain == "own" else 1
                        cur = up.tile([128, 2048], u_dt, tag="u", name="u",
                                      bufs=3)
                        nc.scalar.activation(
                            out=cur[:], in_=ps[:],
                            func=mybir.ActivationFunctionType.Exp,
                            scale=sc[:, sci:sci + 1],
                            accum_out=rid[:, 0:1])
                        scr = up.tile([128, 2048], u_dt, tag="usq",
                                      name="usq", bufs=2)
                        nc.vector._custom_dve(
                            mmd_op, out=scr[:], in0=cur[:],
                            accum_out=rid[:, 1:2])

            for u in range(NUNIT):
                nc.sync.dma_start(out=rid_d.ap()[u],
                                  in_=riders[nrep - 1][u][:])

    nc.compile()
    return nc


_PROG = None


def _get_program():
    global _PROG
    if _PROG is None:
        _PROG = _build_program()
    return _PROG


def _prep_inputs(latent):
    X = np.asarray(latent, np.float32)
    X64 = X.astype(np.float64)
    sq = (X64 * X64).sum(1)                      # [N]
    M2 = float(N) * N - N

    def block_d2_sum(lo, hi):
        n = hi - lo
        sv = X64[lo:hi].sum(0)
        return 2.0 * (n * sq[lo:hi].sum()) - 2.0 * (sv @ sv)

    S_src = block_d2_sum(0, HALF)
    S_tgt = block_d2_sum(HALF, N)
    sv_all = X64.sum(0)
    S_full = 2.0 * (N * sq.sum()) - 2.0 * (sv_all @ sv_all)

    bw_xx = S_src / M2           # already includes /mul^(num//2) (see notes)
    bw_yy = S_tgt / M2
    bw_xy = (S_full / M2) / 4.0

    in_maps = []
    for core in range(NCORES):
        lc = _local_cols(core)
        xf = X[lc].T.reshape(4, 128, NCHUNK * 512)
        xth = np.ascontiguousarray(xf).astype(ml_dtypes.bfloat16)
        sql = sq[lc]
        v = -0.5 * sql
        hi = np.asarray(v, ml_dtypes.bfloat16).astype(np.float64)
        lo = (v - hi).astype(np.float32)
        hi = hi.astype(np.float32)
        ones = np.ones_like(hi)
        aug = np.zeros((4, NCHUNK * 512 + 512), ml_dtypes.bfloat16)
        aug[0, :NCHUNK * 512] = hi
        aug[1, :NCHUNK * 512] = lo
        aug[2, :NCHUNK * 512] = ones
        aug[3, :NCHUNK * 512] = ones
        aug[0, NCHUNK * 512:] = 1.0
        aug[1, NCHUNK * 512:] = 1.0
        aug[2, NCHUNK * 512:] = hi[:512]
        aug[3, NCHUNK * 512:] = lo[:512]

        bw_own = bw_xx if core < 4 else bw_yy
        scales = np.zeros((128, 2), np.float32)
        scales[:, 0] = 1.0 / (8.0 * bw_own)
        scales[:, 1] = 1.0 / (8.0 * bw_xy)
        in_maps.append({"xth": xth, "aug": aug, "scales": scales})
    return in_maps


def _postprocess(results):
    S_own = np.zeros(NCORES)
    S_xy = np.zeros(NCORES)
    for core in range(NCORES):
        r = results[core]["riders"].astype(np.float64)  # [NUNIT,128,RID_W]
        for u, (c, chain) in enumerate(SCHED):
            val = CHUNK_W[c] * r[u].sum()
            if chain == "own":
                S_own[core] += val
            else:
                S_xy[core] += val
    xx = S_own[:4].sum() / (HALF * HALF)
    yy = S_own[4:].sum() / (HALF * HALF)
    xy = S_xy.sum() / (float(N) * N)
    return np.float32(xx + yy - 2.0 * xy)


def _run(inputs, trace=False, **kw):
    from concourse.bass_utils import run_bass_kernel_spmd
    nc = _get_program()
    in_maps = _prep_inputs(inputs["latent"])
    res = run_bass_kernel_spmd(nc, in_maps, list(range(NCORES)),
                               trace=trace, **kw)
    return _postprocess(res.results), res


def kernel(**inputs):
    out, _ = _run(inputs, trace=False)
    return out


if __name__ == "__main__":
    rng = np.random.default_rng(0)
    lat = rng.standard_normal((N, D)).astype(np.float32)
    print(kernel(latent=lat,
                 domain=np.concatenate([np.zeros(HALF, np.int32),
                                        np.ones(HALF, np.int32)])))
